# revision 2
# baseline (speedup 1.0000x reference)
"""Trainium2 Bass kernel for nn_CopulaDecoder.

Data-parallel over batch: core b computes batch element b end-to-end.
All activations live transposed (features on partitions, tokens on free dim).
The neighbor-gather softmax is reformulated as a dense count-matrix softmax:
  softmax over the 64 gathered scores == (C * exp(scale*S)) normalized, where
  C[p,v] = sum_n 1[neighbor_index[p,n]==v] * exp(-scale*attn_mask[p,n]).
Scores are small (|scale*S| < ~4 for this model family), so no max-shift.

Precision: bf16 matmuls for the big KV-MLP + attention (final rel err ~7e-4),
bf16x2 (hi+lo split, 3 accumulated bf16 matmuls) for the small sensitive
matmuls (ds/ff/decoder), fp32 accumulate everywhere, fp32 elementwise.

Scheduling structure (v2): emission order interleaves the two layers so the
attention phase (ACT exp + count-multiply heavy) overlaps the other layer's
KV-MLP matmuls, keeping the PE dense and HAM-warm. All PSUM tiles are
[128,512] (1 bank) in a 6-deep rotating ring + 2-deep AV ring. The true_u
rank-1 (K=1) matmuls are 2-way row-packed; the k-side mm3 (M=32) is 4-way
column-packed. Elementwise work alternates Scalar/Vector; the count-matrix
multiply runs on the otherwise-idle GpSimd engine.
"""
import os

import numpy as np
import ml_dtypes

B, S, T = 8, 32, 64
V = S * T
P = 512
N = 2 * S
I = 256
H, AD = 8, 32
D = H * AD
M = 256
L = 2
R = 128
SCALE = float(AD) ** -0.5

BF = ml_dtypes.bfloat16

_BUILT = {}


# ---------------------------------------------------------------------------
# walrus wait-slot workaround (inlined; see dev notes): Tile attaches >1
# semaphore wait to one instruction; many ISA encodings have a single wait
# slot.  Peel excess waits onto injected same-engine InstNoOps.
# ---------------------------------------------------------------------------
def _install_waitfix():
    import bass_rust
    import concourse.mybir as mybir
    import concourse.tile as tile_mod

    if getattr(tile_mod.TileContext, "_waitfix_installed", False):
        return
    limits = {"InstDrain": 1000, "InstEventSemaphore": 1000, "InstCall": 1000}
    counter = [0]
    orig_add = tile_mod.TileContext._add_instruction

    def patched_add(self, inst):
        si = inst.sync_info
        if si is not None:
            limit = limits.get(type(inst).__name__, 1)
            waits = list(si.on_wait)
            if len(waits) > limit:
                keep = waits[-limit:]
                excess = waits[:-limit]
                while excess:
                    chunk, excess = excess[:1], excess[1:]
                    counter[0] += 1
                    nop = bass_rust.InstNoOp(
                        name=f"waitsplit-{counter[0]}", ins=[], outs=[])
                    nop.engine = inst.engine
                    nop.sync_info = mybir.SyncInfo(on_wait=chunk, on_update=[])
                    orig_add(self, nop)
                inst.sync_info = mybir.SyncInfo(
                    on_wait=keep, on_update=list(si.on_update))
        orig_add(self, inst)

    def patched_drain_and_barrier(self, tick_clock, wait_clock):
        from concourse.tile import ScopedClock

        drain_inst = self.nc.sync.drain()
        wait_clock.add_sem_waits(
            drain_inst.ins, ScopedClock({None: tick_clock.global_clock}))
        si = drain_inst.ins.sync_info
        if si is not None and len(si.on_wait) > 1:
            waits = list(si.on_wait)
            drain_inst.ins.sync_info = mybir.SyncInfo(
                on_wait=waits[:1], on_update=list(si.on_update))
            rest = waits[1:]
            while rest:
                chunk, rest = rest[:1], rest[1:]
                nop = self.nc.sync.nop()
                nop.ins.sync_info = mybir.SyncInfo(on_wait=chunk, on_update=[])
        self.nc.all_engine_barrier()
        assert self.sems is not None
        popped = self.nc._tile_sem_poison_stack.pop()
        assert popped is self._sem_poison
        self.nc.clear_and_free_semaphores(list(self.sems.allocated().values()))
        self.nc.all_engine_barrier()

    try:
        import concourse.tile_utils as tile_utils
        tile_utils.max_sbuf_usage = 204 * 1024
    except Exception:
        pass
    tile_mod.TileContext._add_instruction = patched_add
    tile_mod.TileContext._drain_and_barrier = patched_drain_and_barrier
    tile_mod.TileContext._waitfix_installed = True


def _build():
    """Emit the single-core Bass program (SPMD across 8 cores)."""
    import concourse.bass as bass
    import concourse.mybir as mybir
    import concourse.tile as tile

    _install_waitfix()

    F32 = mybir.dt.float32
    BF16 = mybir.dt.bfloat16
    AF = mybir.ActivationFunctionType
    ALU = mybir.AluOpType

    nc = bass.Bass()

    def din(name, shape, dt=BF16):
        return nc.dram_tensor(name, list(shape), dt, kind="ExternalInput")

    # --- DRAM inputs -------------------------------------------------------
    xt0 = din("xt0", [128, V])            # merged.T rows 0:128      (bf16)
    xt1 = din("xt1", [128, V])            # rows 128:256
    xt2 = din("xt2", [1, V])              # row 256 (true_u)
    ctm = din("ctm", [V, P])              # count matrix transposed  (bf16)
    curh = din("curh", [I, P])            # cur.T hi                 (bf16)
    curl = din("curl", [I, P])            # cur.T lo
    updc = din("updc", [P, 1], F32)       # true_u at pred points    (f32)

    kvw = {}
    for pre in ("k", "v"):
        kvw[pre + "1"] = din(pre + "w1", [L, H, I + 1, M])
        kvw[pre + "2"] = din(pre + "w2", [L, H, M, M])
        kvw[pre + "3"] = din(pre + "w3", [L, H, M, AD])
    kb1d = din("kb1d", [L, H, 2, 128], mybir.dt.float32)
    kb2d = din("kb2d", [L, H, 2, 128], mybir.dt.float32)
    kb3d = din("kb3d", [L, H, AD, 1], mybir.dt.float32)
    vb1d = din("vb1d", [L, H, 2, 128], mybir.dt.float32)
    vb2d = din("vb2d", [L, H, 2, 128], mybir.dt.float32)
    vb3r = din("vb3r", [L, H, 1, P])      # vb3 tiled 16x            (bf16)

    dswh = din("dswh", [I, D]); dswl = din("dswl", [I, D])
    dsbh = din("dsbh", [1, 128 * 2]); dsbl = din("dsbl", [1, 128 * 2])
    ffw1h = din("ffw1h", [L, D, D]); ffw1l = din("ffw1l", [L, D, D])
    ffw2h = din("ffw2h", [L, D, D]); ffw2l = din("ffw2l", [L, D, D])
    ffb1h = din("ffb1h", [L, 1, D]); ffb1l = din("ffb1l", [L, 1, D])
    ffb2h = din("ffb2h", [L, 1, D]); ffb2l = din("ffb2l", [L, 1, D])
    ln1gd = din("ln1gd", [L, 2, 128, 1], F32)
    ln1bd = din("ln1bd", [L, 2, 128, 1], F32)
    ln2gd = din("ln2gd", [L, 2, 128, 1], F32)
    ln2bd = din("ln2bd", [L, 2, 128, 1], F32)
    dew1h = din("dew1h", [D, M]); dew1l = din("dew1l", [D, M])
    dew2h = din("dew2h", [M, M]); dew2l = din("dew2l", [M, M])
    dew3h = din("dew3h", [M, R]); dew3l = din("dew3l", [M, R])
    deb1h = din("deb1h", [1, M]); deb1l = din("deb1l", [1, M])
    deb2h = din("deb2h", [1, M]); deb2l = din("deb2l", [1, M])
    deb3h = din("deb3h", [1, R]); deb3l = din("deb3l", [1, R])

    oh8d = din("oh8d", [8, D], F32)       # onehot head->rows for recip bcast
    iotad = din("iotad", [128, R], F32)   # iota[p, j] = j

    out_d = nc.dram_tensor("out", [1, 1], F32, kind="ExternalOutput")

    with tile.TileContext(nc) as tc:
        with (
            tc.tile_pool(name="const", bufs=1) as cpool,
            tc.tile_pool(name="resident", bufs=1) as rpool,
            tc.tile_pool(name="wts", bufs=2) as wpool,
            tc.tile_pool(name="work", bufs=1) as kpool,
            tc.tile_pool(name="psum", bufs=1, space="PSUM") as pp,
        ):
            # --- constants / resident tensors ---------------------------
            ones_r128 = cpool.tile([1, 128], BF16, name="ones_r128")
            nc.vector.memset(ones_r128[:], 1.0)
            ones_r512 = cpool.tile([1, 512], BF16, name="ones_r512")
            nc.vector.memset(ones_r512[:], 1.0)
            ones_c128b = cpool.tile([128, 1], BF16, name="ones_c128b")
            nc.vector.memset(ones_c128b[:], 1.0)
            ones_c128f = cpool.tile([128, 1], F32, name="ones_c128f")
            nc.vector.memset(ones_c128f[:], 1.0)
            ones_rbf = cpool.tile([1, 128], F32, name="ones_rbf")
            nc.vector.memset(ones_rbf[:], 1.0)
            eps_t = cpool.tile([1, 1], F32, name="eps_t")
            nc.vector.memset(eps_t[:], 1e-5)
            nlogr_t = cpool.tile([1, 1], F32, name="nlogr_t")
            nc.vector.memset(nlogr_t[:], -float(P) * float(np.log(R)))
            oh8 = cpool.tile([8, D], F32, name="oh8")
            nc.sync.dma_start(oh8[:], oh8d[:])
            iota = cpool.tile([128, R], F32, name="iota")
            nc.sync.dma_start(iota[:], iotad[:])

            xt = [rpool.tile([128, V], BF16, name=f"xt{i}") for i in range(2)]
            nc.sync.dma_start(xt[0][:], xt0[:])
            nc.sync.dma_start(xt[1][:], xt1[:])
            # u replicated at partitions 0 and 32 for 2-way row-packed
            # K=1 matmuls (true_u rank-1 term of mm1)
            u2 = rpool.tile([64, V], BF16, name="u2")
            nc.sync.dma_start(u2[0:1, :], xt2[:])
            nc.sync.dma_start(u2[32:33, :], xt2[:])

            ct = rpool.tile([128, 16, P], BF16, name="ct")
            nc.sync.dma_start(
                ct[:], ctm.rearrange("(c p) q -> p c q", p=128))

            cur_h = [kpool.tile([128, P], BF16, tag=f"cur_h{q}", bufs=1,
                                name=f"cur_h{q}") for q in range(2)]
            cur_l = [kpool.tile([128, P], BF16, tag=f"cur_l{q}", bufs=1,
                                name=f"cur_l{q}") for q in range(2)]
            for q in range(2):
                nc.sync.dma_start(cur_h[q][:], curh[128 * q:128 * (q + 1), :])
                nc.sync.dma_start(cur_l[q][:], curl[128 * q:128 * (q + 1), :])

            # keys (transposed, per (l, quad)) and vals (+ones, per (l,h))
            kt = [[rpool.tile([128, V], BF16, name=f"kt{l}{q}")
                   for q in range(2)] for l in range(L)]
            vals = [[rpool.tile([128, 16, AD + 1], BF16, name=f"vals{l}{h}")
                     for h in range(H)] for l in range(L)]

            # Rotating psum rings: all tiles [*,512] one bank.
            def ps_tile(nm, p=128):
                return pp.tile([p, 512], mybir.dt.float32, tag="ps",
                               name=nm, bufs=6, uniquify=True)

            def psA_tile(nm):
                return pp.tile([128, 512], mybir.dt.float32, tag="psA",
                               name=nm, bufs=2, uniquify=True)

            # elementwise engine alternation (scalar <-> vector)
            ew_state = [0]

            def relu_out(dst, src_ps, bias):
                """dst = relu(src_ps + bias), alternating ACT/DVE."""
                ew_state[0] ^= 1
                if ew_state[0]:
                    nc.scalar.activation(dst, src_ps, AF.Relu, bias=bias)
                else:
                    nc.vector.tensor_scalar(dst, src_ps, bias, 0.0,
                                            ALU.add, ALU.max)

            # =============================================================
            # Phase helpers
            # =============================================================
            def kv_chain(l, h, pre):
                """One (layer, head, k-or-v) MLP chain over all V rows."""
                w1d, w2d, w3d = kvw[pre + "1"], kvw[pre + "2"], kvw[pre + "3"]
                cn = f"{pre}{l}{h}"
                w1a = wpool.tile([128, M], BF16, tag="w1a", name=f"w1a{cn}")
                w1b = wpool.tile([128, M], BF16, tag="w1b", name=f"w1b{cn}")
                w1c2 = wpool.tile([64, M], BF16, tag="w1c2", name=f"w1c2{cn}")
                w2a = wpool.tile([128, M], BF16, tag="w2a", name=f"w2a{cn}")
                w2b = wpool.tile([128, M], BF16, tag="w2b", name=f"w2b{cn}")
                w3a = wpool.tile([128, AD], BF16, tag="w3a", name=f"w3a{cn}")
                w3b = wpool.tile([128, AD], BF16, tag="w3b", name=f"w3b{cn}")
                nc.sync.dma_start(w1a[:], w1d[l, h, 0:128, :])
                nc.sync.dma_start(w1b[:], w1d[l, h, 128:256, :])
                nc.sync.dma_start(w1c2[0:1, :], w1d[l, h, 256:257, :])
                nc.sync.dma_start(w1c2[32:33, :], w1d[l, h, 256:257, :])
                nc.sync.dma_start(w2a[:], w2d[l, h, 0:128, :])
                nc.sync.dma_start(w2b[:], w2d[l, h, 128:256, :])
                nc.sync.dma_start(w3a[:], w3d[l, h, 0:128, :])
                nc.sync.dma_start(w3b[:], w3d[l, h, 128:256, :])
                b1d = kb1d if pre == "k" else vb1d
                b2d = kb2d if pre == "k" else vb2d
                b1 = [wpool.tile([128, 1], mybir.dt.float32, tag=f"b1_{fc}",
                                 name=f"b1{cn}_{fc}") for fc in range(2)]
                b2 = [wpool.tile([128, 1], mybir.dt.float32, tag=f"b2_{fc}",
                                 name=f"b2{cn}_{fc}") for fc in range(2)]
                for fc in range(2):
                    nc.sync.dma_start(
                        b1[fc][:], b1d[l, h, fc, :].rearrange("(p o) -> p o", o=1))
                    nc.sync.dma_start(
                        b2[fc][:], b2d[l, h, fc, :].rearrange("(p o) -> p o", o=1))

                # per-nt pipeline: mm1 (both fc, u 2-way row-packed) -> relu
                # -> mm2 -> relu; h2 tiles kept for mm3
                h2t = {}
                for nt in range(4):
                    sl = slice(512 * nt, 512 * (nt + 1))
                    ps1 = [ps_tile(f"ps1{cn}{fc}{nt}") for fc in range(2)]
                    # rank-1 true_u term: K=1 matmuls at row tiles 0 / 32
                    nc.tensor.matmul(
                        ps1[0][:], w1c2[0:1, 0:128], u2[0:1, sl],
                        start=True, stop=False)
                    nc.tensor.matmul(
                        ps1[1][:], w1c2[32:33, 128:256], u2[32:33, sl],
                        start=True, stop=False, tile_position=(32, 0))
                    for fc in range(2):
                        cs = slice(128 * fc, 128 * (fc + 1))
                        nc.tensor.matmul(ps1[fc][:], w1a[:, cs], xt[0][:, sl],
                                         start=False, stop=False)
                        nc.tensor.matmul(ps1[fc][:], w1b[:, cs], xt[1][:, sl],
                                         start=False, stop=True)
                    h1 = []
                    for fc in range(2):
                        t = kpool.tile([128, 512], BF16, tag="h1",
                                       name=f"h1{cn}{fc}{nt}", bufs=6,
                                       uniquify=True)
                        relu_out(t[:], ps1[fc][:], b1[fc][:])
                        h1.append(t)
                    for fc in range(2):
                        cs = slice(128 * fc, 128 * (fc + 1))
                        ps2 = ps_tile(f"ps2{cn}{fc}{nt}")
                        nc.tensor.matmul(ps2[:], w2a[:, cs], h1[0][:],
                                         start=True, stop=False)
                        nc.tensor.matmul(ps2[:], w2b[:, cs], h1[1][:],
                                         start=False, stop=True)
                        t = kpool.tile([128, 512], BF16, tag="h2",
                                       name=f"h2{cn}{fc}{nt}", bufs=10,
                                       uniquify=True)
                        relu_out(t[:], ps2[:], b2[fc][:])
                        h2t[(fc, nt)] = t

                if pre == "k":
                    b3 = wpool.tile([AD, 1], mybir.dt.float32, tag="b3",
                                    name=f"b3{cn}")
                    nc.sync.dma_start(b3[:], kb3d[l, h, :, :])
                    q, hp = h // 4, h % 4
                    # 4-way column-packed: col tile s covers keys quarter s
                    psk = ps_tile(f"psk{cn}")
                    for s in range(4):
                        nc.tensor.matmul(psk[32 * s:32 * (s + 1), :],
                                         w3a[:], h2t[(0, s)][:],
                                         start=True, stop=False,
                                         tile_position=(0, 32 * s))
                        nc.tensor.matmul(psk[32 * s:32 * (s + 1), :],
                                         w3b[:], h2t[(1, s)][:],
                                         start=False, stop=True,
                                         tile_position=(0, 32 * s))
                    for s in range(4):
                        nc.vector.tensor_scalar(
                            kt[l][q][32 * hp:32 * (hp + 1),
                                     512 * s:512 * (s + 1)],
                            psk[32 * s:32 * (s + 1), :], b3[:], None, ALU.add)
                else:
                    vb3t = wpool.tile([1, P], BF16, tag="vb3t", name=f"vb3t{cn}")
                    nc.sync.dma_start(vb3t[:], vb3r[l, h, :, :])
                    psv = ps_tile(f"psv{cn}")
                    nc.tensor.matmul(psv[:], ones_r128[:], vb3t[:],
                                     start=True, stop=False)
                    for svg in range(16):
                        nt, c = svg // 4, svg % 4
                        vsl = slice(128 * c, 128 * (c + 1))
                        osl = slice(32 * svg, 32 * (svg + 1))
                        last = (svg == 15)
                        nc.tensor.matmul(
                            psv[:, osl], h2t[(0, nt)][:, vsl], w3a[:],
                            start=False, stop=False)
                        nc.tensor.matmul(
                            psv[:, osl], h2t[(1, nt)][:, vsl], w3b[:],
                            start=False, stop=last)
                    vt = vals[l][h]
                    nc.vector.tensor_copy(
                        vt[:, :, 0:AD],
                        psv[:].rearrange("p (s d) -> p s d", d=AD))
                    nc.vector.memset(vt[:, :, AD:AD + 1], 1.0)

            # attv tiles (f32) + bf16/lo splits, rotated per layer
            def split_bf(src_tiles, tagp, need_lo=True):
                """f32 [128,P] tiles -> (hi bf16, lo bf16) tiles."""
                his, los = [], []
                for q, s in enumerate(src_tiles):
                    hi = kpool.tile([128, P], BF16, tag=f"{tagp}h{q}",
                                    name=f"{tagp}h{q}", bufs=2, uniquify=True)
                    nc.vector.tensor_copy(hi[:], s[:])
                    his.append(hi)
                    if need_lo:
                        lo = kpool.tile([128, P], BF16, tag=f"{tagp}l{q}",
                                        name=f"{tagp}l{q}", bufs=2, uniquify=True)
                        nc.vector.tensor_tensor(lo[:], s[:], hi[:], ALU.subtract)
                        los.append(lo)
                return his, los

            def mm6(ps, lhsh, lhsl, rhsh, rhsl, start, stop=False):
                """bf16x2 product accumulate: hh + lh + hl."""
                nc.tensor.matmul(ps, lhsh, rhsh, start=start, stop=False)
                nc.tensor.matmul(ps, lhsl, rhsh, start=False, stop=False)
                nc.tensor.matmul(ps, lhsh, rhsl, start=False, stop=stop)

            def bias_mm(ps, bh, bl):
                """T-layout +bias: lhsT = bias chunk [1,128] (M=feat),
                rhs = ones [1,512] (N=tokens)."""
                nc.tensor.matmul(ps, bh, ones_r512[:], start=True, stop=False)
                nc.tensor.matmul(ps, bl, ones_r512[:], start=False, stop=False)

            def layer_norm(xq, gd, bd, l, nm):
                """T-layout LN over 256 features; returns new f32 tiles."""
                xh, _ = split_bf(xq, "lnx", need_lo=False)
                pst = ps_tile(f"lnsum{nm}")
                nc.tensor.matmul(pst[0:1, :], ones_c128b[:], xh[0][:],
                                 start=True, stop=False)
                nc.tensor.matmul(pst[0:1, :], ones_c128b[:], xh[1][:],
                                 start=False, stop=True)
                sq = [kpool.tile([128, P], BF16, tag=f"lnsq{q}",
                                 name=f"lnsq{nm}{q}", bufs=1) for q in range(2)]
                for q in range(2):
                    nc.vector.tensor_tensor(sq[q][:], xh[q][:], xh[q][:], ALU.mult)
                psq = ps_tile(f"lnsq{nm}")
                nc.tensor.matmul(psq[0:1, :], ones_c128b[:], sq[0][:],
                                 start=True, stop=False)
                nc.tensor.matmul(psq[0:1, :], ones_c128b[:], sq[1][:],
                                 start=False, stop=True)
                mu = kpool.tile([1, P], mybir.dt.float32, tag="lnmu", bufs=1,
                                name=f"lnmu{nm}")
                nc.scalar.mul(mu[:], pst[0:1, :], 1.0 / D)
                m2 = kpool.tile([1, P], mybir.dt.float32, tag="lnm2", bufs=1,
                                name=f"lnm2{nm}")
                nc.vector.tensor_tensor(m2[:], mu[:], mu[:], ALU.mult)
                var = kpool.tile([1, P], mybir.dt.float32, tag="lnvar", bufs=1,
                                 name=f"lnvar{nm}")
                nc.vector.scalar_tensor_tensor(
                    var[:], psq[0:1, :], 1.0 / D, m2[:], ALU.mult, ALU.subtract)
                sd = kpool.tile([1, P], mybir.dt.float32, tag="lnsd", bufs=1,
                                name=f"lnsd{nm}")
                nc.scalar.activation(sd[:], var[:], AF.Sqrt, bias=eps_t[:])
                rstd = kpool.tile([1, P], mybir.dt.float32, tag="lnrs", bufs=1,
                                  name=f"lnrs{nm}")
                nc.vector.reciprocal(rstd[:], sd[:])
                nmu = kpool.tile([1, P], mybir.dt.float32, tag="lnnm", bufs=1,
                                 name=f"lnnm{nm}")
                nc.vector.scalar_tensor_tensor(
                    nmu[:], mu[:], -1.0, rstd[:], ALU.mult, ALU.mult)
                # broadcast A=rstd, B=-mu*rstd to 128 partitions (fp32 matmul)
                psa = ps_tile(f"lnA{nm}")
                nc.tensor.matmul(psa[:], ones_rbf[:], rstd[:], start=True, stop=True)
                psb = ps_tile(f"lnB{nm}")
                nc.tensor.matmul(psb[:], ones_rbf[:], nmu[:], start=True, stop=True)
                outq = []
                for q in range(2):
                    g = wpool.tile([128, 1], mybir.dt.float32, tag=f"lng{q}",
                                   name=f"lng{nm}{q}")
                    nc.sync.dma_start(g[:], gd[l, q, :, :])
                    bb = wpool.tile([128, 1], mybir.dt.float32, tag=f"lnb{q}",
                                    name=f"lnb{nm}{q}")
                    nc.sync.dma_start(bb[:], bd[l, q, :, :])
                    t1 = kpool.tile([128, P], mybir.dt.float32, tag=f"lnt{q}",
                                    name=f"lnt{nm}{q}", bufs=1)
                    nc.vector.tensor_tensor(t1[:], xq[q][:], psa[:], ALU.mult)
                    nc.vector.tensor_tensor(t1[:], t1[:], psb[:], ALU.add)
                    o = kpool.tile([128, P], mybir.dt.float32, tag=f"attv{q}",
                                   name=f"ln_out{nm}{q}", bufs=2)
                    nc.vector.tensor_scalar(o[:], t1[:], g[:], bb[:],
                                            ALU.mult, ALU.add)
                    outq.append(o)
                return outq

            def attention(l, attv):
                """Count-matrix softmax attention; returns residual+LN out."""
                qt_h, _ = split_bf(attv, "qt", need_lo=False)
                numer = [kpool.tile([128, P], mybir.dt.float32, tag=f"num{q}",
                                    name=f"numer{l}{q}", bufs=1)
                         for q in range(2)]
                dn8 = kpool.tile([8, P], mybir.dt.float32, tag="dn8",
                                 name=f"dn8{l}", bufs=1)
                for pair in range(4):
                    h0 = 2 * pair
                    h1 = h0 + 1
                    q = h0 // 4
                    b0, b1r = 32 * (h0 % 4), 32 * (h1 % 4)
                    psA = psA_tile(f"psA{l}{pair}")
                    for vc in range(16):
                        ces = {}
                        for hh, bb in ((h0, b0), (h1, b1r)):
                            pss = ps_tile(f"pss{l}{hh}{vc}")
                            tp = (bb, 0) if bb >= 64 else None
                            nc.tensor.matmul(
                                pss[:],
                                kt[l][q][bb:bb + 32, 128 * vc:128 * (vc + 1)],
                                qt_h[q][bb:bb + 32, :],
                                start=True, stop=True, tile_position=tp)
                            e = kpool.tile([128, 512], BF16, tag="ebuf",
                                           name=f"e{l}{hh}{vc}", bufs=4,
                                           uniquify=True)
                            nc.scalar.activation(e[:], pss[:], AF.Exp,
                                                 scale=SCALE)
                            ce = kpool.tile([128, 512], BF16, tag="cebuf",
                                            name=f"ce{l}{hh}{vc}", bufs=4,
                                            uniquify=True)
                            nc.gpsimd.tensor_tensor(
                                ce[:], e[:], ct[:, vc, :], ALU.mult)
                            ces[hh] = ce
                        nc.tensor.matmul(
                            psA[0:AD + 1, :], vals[l][h0][:, vc, :],
                            ces[h0][:], start=(vc == 0), stop=(vc == 15))
                        nc.tensor.matmul(
                            psA[64:64 + AD + 1, :], vals[l][h1][:, vc, :],
                            ces[h1][:], start=(vc == 0), stop=(vc == 15),
                            tile_position=(0, 64))
                    nc.vector.tensor_copy(numer[q][b0:b0 + 32, :],
                                          psA[0:32, :])
                    nc.vector.tensor_copy(numer[q][b1r:b1r + 32, :],
                                          psA[64:96, :])
                    for hh, prow in ((h0, 32), (h1, 96)):
                        dtmp = kpool.tile([1, P], mybir.dt.float32,
                                          tag="dntmp", bufs=2,
                                          name=f"dtmp{l}{hh}", uniquify=True)
                        nc.vector.tensor_copy(dtmp[:], psA[prow:prow + 1, :])
                        nc.sync.dma_start(dn8[hh:hh + 1, :], dtmp[:])

                rd8 = kpool.tile([8, P], mybir.dt.float32, tag="rd8",
                                 name=f"rd8{l}", bufs=1)
                nc.vector.reciprocal(rd8[:], dn8[:])
                xres = []
                for q in range(2):
                    psrb = ps_tile(f"psrb{l}{q}")
                    nc.tensor.matmul(psrb[:], oh8[:, 128 * q:128 * (q + 1)],
                                     rd8[:], start=True, stop=True)
                    t1 = kpool.tile([128, P], mybir.dt.float32, tag=f"xres{q}",
                                    name=f"xres{l}{q}", bufs=1)
                    nc.vector.tensor_tensor(t1[:], numer[q][:], psrb[:], ALU.mult)
                    nc.vector.tensor_tensor(t1[:], attv[q][:], t1[:], ALU.add)
                    xres.append(t1)
                return layer_norm(xres, ln1gd, ln1bd, l, f"ln1_{l}")

            def ff_block(l, attv):
                fw1h = [wpool.tile([128, D], BF16, tag=f"fw1h{kc}",
                                   name=f"fw1h{l}{kc}") for kc in range(2)]
                fw1l = [wpool.tile([128, D], BF16, tag=f"fw1l{kc}",
                                   name=f"fw1l{l}{kc}") for kc in range(2)]
                fw2h = [wpool.tile([128, D], BF16, tag=f"fw2h{kc}",
                                   name=f"fw2h{l}{kc}") for kc in range(2)]
                fw2l = [wpool.tile([128, D], BF16, tag=f"fw2l{kc}",
                                   name=f"fw2l{l}{kc}") for kc in range(2)]
                for kc in range(2):
                    sl = slice(128 * kc, 128 * (kc + 1))
                    nc.sync.dma_start(fw1h[kc][:], ffw1h[l, sl, :])
                    nc.sync.dma_start(fw1l[kc][:], ffw1l[l, sl, :])
                    nc.sync.dma_start(fw2h[kc][:], ffw2h[l, sl, :])
                    nc.sync.dma_start(fw2l[kc][:], ffw2l[l, sl, :])
                fb1h = wpool.tile([1, D], BF16, tag="fb1h", name=f"fb1h{l}")
                fb1l = wpool.tile([1, D], BF16, tag="fb1l", name=f"fb1l{l}")
                fb2h = wpool.tile([1, D], BF16, tag="fb2h", name=f"fb2h{l}")
                fb2l = wpool.tile([1, D], BF16, tag="fb2l", name=f"fb2l{l}")
                nc.sync.dma_start(fb1h[:], ffb1h[l, :, :])
                nc.sync.dma_start(fb1l[:], ffb1l[l, :, :])
                nc.sync.dma_start(fb2h[:], ffb2h[l, :, :])
                nc.sync.dma_start(fb2l[:], ffb2l[l, :, :])

                av_h, av_l = split_bf(attv, "ffs")
                hh_t, hl_t = [], []
                for fc in range(2):
                    psf = ps_tile(f"psff1{l}{fc}")
                    bias_mm(psf[:], fb1h[:, 128 * fc:128 * (fc + 1)],
                            fb1l[:, 128 * fc:128 * (fc + 1)])
                    for kc in range(2):
                        mm6(psf[:], fw1h[kc][:, 128 * fc:128 * (fc + 1)],
                            fw1l[kc][:, 128 * fc:128 * (fc + 1)],
                            av_h[kc][:], av_l[kc][:], start=False, stop=(kc == 1))
                    th = kpool.tile([128, P], BF16, tag=f"ffhh{fc}",
                                    name=f"ffhh{l}{fc}", bufs=2)
                    nc.scalar.activation(th[:], psf[:], AF.Relu)
                    tl = kpool.tile([128, P], BF16, tag=f"ffhl{fc}",
                                    name=f"ffhl{l}{fc}", bufs=2)
                    nc.vector.scalar_tensor_tensor(
                        tl[:], psf[:], 0.0, th[:], ALU.max, ALU.subtract)
                    hh_t.append(th); hl_t.append(tl)
                xres2 = []
                for fc in range(2):
                    psf2 = ps_tile(f"psff2{l}{fc}")
                    bias_mm(psf2[:], fb2h[:, 128 * fc:128 * (fc + 1)],
                            fb2l[:, 128 * fc:128 * (fc + 1)])
                    for kc in range(2):
                        mm6(psf2[:], fw2h[kc][:, 128 * fc:128 * (fc + 1)],
                            fw2l[kc][:, 128 * fc:128 * (fc + 1)],
                            hh_t[kc][:], hl_t[kc][:], start=False, stop=(kc == 1))
                    t2 = kpool.tile([128, P], mybir.dt.float32, tag=f"xres{fc}",
                                    name=f"xr2{l}{fc}", bufs=1)
                    nc.vector.tensor_tensor(t2[:], attv[fc][:], psf2[:], ALU.add)
                    xres2.append(t2)
                return layer_norm(xres2, ln2gd, ln2bd, l, f"ln2_{l}")

            # =============================================================
            # Emit program
            # =============================================================
            # ds projection: attv0 = cur @ ds_W + ds_b   (T-layout out)
            dsw_h = [cpool.tile([128, D], BF16, name=f"dswh{kc}") for kc in range(2)]
            dsw_l = [cpool.tile([128, D], BF16, name=f"dswl{kc}") for kc in range(2)]
            for kc in range(2):
                nc.sync.dma_start(dsw_h[kc][:], dswh[128 * kc:128 * (kc + 1), :])
                nc.sync.dma_start(dsw_l[kc][:], dswl[128 * kc:128 * (kc + 1), :])
            dsb_h = cpool.tile([1, 256], BF16, name="dsb_h")
            nc.sync.dma_start(dsb_h[:], dsbh[:])
            dsb_l = cpool.tile([1, 256], BF16, name="dsb_l")
            nc.sync.dma_start(dsb_l[:], dsbl[:])

            attv = []
            for q in range(2):
                psd = ps_tile(f"psds{q}")
                bias_mm(psd[:], dsb_h[:, 128 * q:128 * (q + 1)],
                        dsb_l[:, 128 * q:128 * (q + 1)])
                for kc in range(2):
                    mm6(psd[:], dsw_h[kc][:, 128 * q:128 * (q + 1)],
                        dsw_l[kc][:, 128 * q:128 * (q + 1)],
                        cur_h[kc][:], cur_l[kc][:], start=False, stop=(kc == 1))
                o = kpool.tile([128, P], mybir.dt.float32, tag=f"attv{q}",
                               name=f"attv0{q}", bufs=2)
                nc.vector.tensor_copy(o[:], psd[:])
                attv.append(o)

            # Layer-interleaved emission: attention phases overlap the other
            # layer's KV-MLP chains via the dataflow scheduler.
            for q in range(2):
                for hp in range(4):
                    kv_chain(0, 4 * q + hp, "k")
            for q in range(2):
                for hp in range(4):
                    kv_chain(0, 4 * q + hp, "v")
            attv = attention(0, attv)
            for q in range(2):
                for hp in range(4):
                    kv_chain(1, 4 * q + hp, "k")
            attv = ff_block(0, attv)
            for q in range(2):
                for hp in range(4):
                    kv_chain(1, 4 * q + hp, "v")
            attv = attention(1, attv)
            attv = ff_block(1, attv)

            # ---- decoder --------------------------------------------------
            dw1h = [cpool.tile([128, M], BF16, name=f"dw1h{kc}") for kc in range(2)]
            dw1l = [cpool.tile([128, M], BF16, name=f"dw1l{kc}") for kc in range(2)]
            dw2h = [cpool.tile([128, M], BF16, name=f"dw2h{kc}") for kc in range(2)]
            dw2l = [cpool.tile([128, M], BF16, name=f"dw2l{kc}") for kc in range(2)]
            dw3h = [cpool.tile([128, R], BF16, name=f"dw3h{kc}") for kc in range(2)]
            dw3l = [cpool.tile([128, R], BF16, name=f"dw3l{kc}") for kc in range(2)]
            for kc in range(2):
                sl = slice(128 * kc, 128 * (kc + 1))
                nc.sync.dma_start(dw1h[kc][:], dew1h[sl, :])
                nc.sync.dma_start(dw1l[kc][:], dew1l[sl, :])
                nc.sync.dma_start(dw2h[kc][:], dew2h[sl, :])
                nc.sync.dma_start(dw2l[kc][:], dew2l[sl, :])
                nc.sync.dma_start(dw3h[kc][:], dew3h[sl, :])
                nc.sync.dma_start(dw3l[kc][:], dew3l[sl, :])
            db1h = cpool.tile([1, M], BF16, name="db1h")
            db1l = cpool.tile([1, M], BF16, name="db1l")
            db2h = cpool.tile([1, M], BF16, name="db2h")
            db2l = cpool.tile([1, M], BF16, name="db2l")
            db3h = cpool.tile([1, R], BF16, name="db3h")
            db3l = cpool.tile([1, R], BF16, name="db3l")
            nc.sync.dma_start(db1h[:], deb1h[:])
            nc.sync.dma_start(db1l[:], deb1l[:])
            nc.sync.dma_start(db2h[:], deb2h[:])
            nc.sync.dma_start(db2l[:], deb2l[:])
            nc.sync.dma_start(db3h[:], deb3h[:])
            nc.sync.dma_start(db3l[:], deb3l[:])

            de_h, de_l = split_bf(attv, "ffs")
            d1h, d1l = [], []
            for fc in range(2):
                psd1 = ps_tile(f"psde1{fc}")
                bias_mm(psd1[:], db1h[:, 128 * fc:128 * (fc + 1)],
                        db1l[:, 128 * fc:128 * (fc + 1)])
                for kc in range(2):
                    mm6(psd1[:], dw1h[kc][:, 128 * fc:128 * (fc + 1)],
                        dw1l[kc][:, 128 * fc:128 * (fc + 1)],
                        de_h[kc][:], de_l[kc][:], start=False, stop=(kc == 1))
                th = kpool.tile([128, P], BF16, tag=f"d1h{fc}",
                                name=f"d1h{fc}", bufs=1)
                nc.scalar.activation(th[:], psd1[:], AF.Relu)
                tl = kpool.tile([128, P], BF16, tag=f"d1l{fc}",
                                name=f"d1l{fc}", bufs=1)
                nc.vector.scalar_tensor_tensor(
                    tl[:], psd1[:], 0.0, th[:], ALU.max, ALU.subtract)
                d1h.append(th); d1l.append(tl)
            d2h, d2l = [], []
            for fc in range(2):
                psd2 = ps_tile(f"psde2{fc}")
                bias_mm(psd2[:], db2h[:, 128 * fc:128 * (fc + 1)],
                        db2l[:, 128 * fc:128 * (fc + 1)])
                for kc in range(2):
                    mm6(psd2[:], dw2h[kc][:, 128 * fc:128 * (fc + 1)],
                        dw2l[kc][:, 128 * fc:128 * (fc + 1)],
                        d1h[kc][:], d1l[kc][:], start=False, stop=(kc == 1))
                th = kpool.tile([128, P], BF16, tag=f"d2h{fc}",
                                name=f"d2h{fc}", bufs=1)
                nc.scalar.activation(th[:], psd2[:], AF.Relu)
                tl = kpool.tile([128, P], BF16, tag=f"d2l{fc}",
                                name=f"d2l{fc}", bufs=1)
                nc.vector.scalar_tensor_tensor(
                    tl[:], psd2[:], 0.0, th[:], ALU.max, ALU.subtract)
                d2h.append(th); d2l.append(tl)

            # logits row-major [p, R] per 128-p chunk + loss
            t4 = kpool.tile([128, 4], mybir.dt.float32, tag="t4",
                            name="t4", bufs=1)
            for pc in range(4):
                psl = ps_tile(f"pslog{pc}")
                nc.tensor.matmul(psl[:, 0:R], ones_r128[:], db3h[:],
                                 start=True, stop=False)
                nc.tensor.matmul(psl[:, 0:R], ones_r128[:], db3l[:],
                                 start=False, stop=False)
                psl_sl = slice(128 * pc, 128 * (pc + 1))
                for kc in range(2):
                    nc.tensor.matmul(psl[:, 0:R], d2h[kc][:, psl_sl], dw3h[kc][:],
                                     start=False, stop=False)
                    nc.tensor.matmul(psl[:, 0:R], d2l[kc][:, psl_sl], dw3h[kc][:],
                                     start=False, stop=False)
                    nc.tensor.matmul(psl[:, 0:R], d2h[kc][:, psl_sl], dw3l[kc][:],
                                     start=False, stop=(kc == 1))
                logit = kpool.tile([128, R], mybir.dt.float32, tag="logit",
                                   name=f"logit{pc}", bufs=2)
                nc.vector.tensor_copy(logit[:], psl[:, 0:R])
                m = kpool.tile([128, 1], mybir.dt.float32, tag="lm",
                               name=f"lm{pc}", bufs=2)
                nc.vector.tensor_reduce(m[:], logit[:], mybir.AxisListType.X,
                                        ALU.max)
                mneg = kpool.tile([128, 1], mybir.dt.float32, tag="lmn",
                                  name=f"lmn{pc}", bufs=2)
                nc.vector.tensor_scalar_mul(mneg[:], m[:], -1.0)
                escr = kpool.tile([128, R], mybir.dt.float32, tag="escr", bufs=1,
                                  name=f"escr{pc}")
                se = kpool.tile([128, 1], mybir.dt.float32, tag="se",
                                name=f"se{pc}", bufs=2)
                nc.scalar.activation(escr[:], logit[:], AF.Exp,
                                     bias=mneg[:], accum_out=se[:])
                ls = kpool.tile([128, 1], mybir.dt.float32, tag="ls",
                                name=f"ls{pc}", bufs=2)
                nc.scalar.activation(ls[:], se[:], AF.Ln)
                upt = kpool.tile([128, 1], mybir.dt.float32, tag="upt",
                                 name=f"upt{pc}", bufs=2)
                nc.sync.dma_start(upt[:], updc[psl_sl, :])
                z = kpool.tile([128, 1], mybir.dt.float32, tag="z",
                               name=f"z{pc}", bufs=2)
                nc.scalar.mul(z[:], upt[:], float(R))
                dm = kpool.tile([128, R], mybir.dt.float32, tag="dm", bufs=1,
                                name=f"dm{pc}")
                nc.vector.tensor_scalar(dm[:], iota[:], -1.0, z[:],
                                        ALU.mult, ALU.add)
                g1 = kpool.tile([128, R], mybir.dt.float32, tag="g1", bufs=1,
                                name=f"g1{pc}")
                nc.vector.tensor_scalar(g1[:], dm[:], 0.0, None, ALU.is_ge)
                g2 = kpool.tile([128, R], mybir.dt.float32, tag="g2", bufs=1,
                                name=f"g2{pc}")
                nc.vector.tensor_scalar(g2[:], dm[:], 1.0, None, ALU.is_lt)
                oh = kpool.tile([128, R], mybir.dt.float32, tag="ohh", bufs=1,
                                name=f"oh{pc}")
                nc.vector.tensor_tensor(oh[:], g1[:], g2[:], ALU.mult)
                scr2 = kpool.tile([128, R], mybir.dt.float32, tag="scr2", bufs=1,
                                  name=f"scr2{pc}")
                pk = kpool.tile([128, 1], mybir.dt.float32, tag="pk",
                                name=f"pk{pc}", bufs=2)
                nc.vector.scalar_tensor_tensor(
                    scr2[:], logit[:], 1.0, oh[:], ALU.mult, ALU.mult,
                    accum_out=pk[:])
                tt = kpool.tile([128, 1], mybir.dt.float32, tag="tt",
                                name=f"tt{pc}", bufs=2)
                nc.vector.tensor_tensor(tt[:], pk[:], m[:], ALU.subtract)
                nc.vector.tensor_tensor(t4[:, pc:pc + 1], tt[:], ls[:],
                                        ALU.subtract)
            pspr = ps_tile("pspr")
            nc.tensor.matmul(pspr[0:1, 0:4], ones_c128f[:], t4[:],
                             start=True, stop=True)
            pr4 = kpool.tile([1, 4], mybir.dt.float32, tag="pr4",
                             name="pr4", bufs=1)
            nc.vector.tensor_copy(pr4[:], pspr[0:1, 0:4])
            s1 = kpool.tile([1, 1], mybir.dt.float32, tag="s1",
                            name="s1", bufs=1)
            nc.vector.tensor_reduce(s1[:], pr4[:], mybir.AxisListType.X, ALU.add)
            outt = kpool.tile([1, 1], mybir.dt.float32, tag="outt",
                              name="outt", bufs=1)
            nc.scalar.activation(outt[:], s1[:], AF.Identity,
                                 bias=nlogr_t[:], scale=-1.0)
            nc.sync.dma_start(out_d[:], outt[:])

    return nc


def _split(x):
    h = np.asarray(x, np.float32).astype(BF)
    lo = (np.asarray(x, np.float32) - h.astype(np.float32)).astype(BF)
    return h, lo


def _maybe_enable_trace():
    """Optional NTFF profiling under axon (KERNEL_TRACE=1); best-effort."""
    try:
        import sys
        import types

        import antenv

        if "antenv.axon_hooks" not in sys.modules:
            mod = types.ModuleType("antenv.axon_hooks")
            mod._hook = None
            mod.set_axon_ntff_profile_hook = lambda h: setattr(mod, "_hook", h)
            mod.get_axon_ntff_profile_hook = lambda: mod._hook
            sys.modules["antenv.axon_hooks"] = mod
            antenv.axon_hooks = mod
            from trn_agent_boot.trn_boot import _ntff_profile_via_ctypes

            mod._hook = _ntff_profile_via_ctypes("/opt/axon/libaxon_pjrt.so")
        import concourse.bass_utils as _bu

        _bu.upload_artifacts = lambda tmpdir: f"file://{tmpdir}"
        return True
    except Exception:
        return False


LAST_RESULT = {}


def kernel(**inputs):
    from concourse.bass_utils import run_bass_kernel_spmd

    if "nc" not in _BUILT:
        _BUILT["nc"] = _build()
    nc = _BUILT["nc"]

    f32 = lambda a: np.ascontiguousarray(np.asarray(a, np.float32))
    bf = lambda a: np.ascontiguousarray(np.asarray(a, np.float32)).astype(BF)

    enc = f32(inputs["encoded"])                      # [B,V,I]
    tu = f32(inputs["true_u"])                        # [B,V,1]
    mask = f32(inputs["attn_mask"])                   # [P,N]
    pp_ = np.asarray(inputs["pred_points"]).astype(np.int64)
    ni = np.asarray(inputs["neighbor_index"]).astype(np.int64)

    # count matrix C[p, v]
    C = np.zeros((P, V), np.float32)
    np.add.at(C, (np.repeat(np.arange(P), N), ni.ravel()),
              np.exp(-SCALE * mask).ravel().astype(np.float32))
    ctm = np.ascontiguousarray(C.T).astype(BF)        # [V, P]

    shared = {"ctm": ctm}
    for pre in ("k", "v"):
        shared[pre + "w1"] = bf(inputs[pre + "W1"])
        shared[pre + "w2"] = bf(inputs[pre + "W2"])
        shared[pre + "w3"] = bf(inputs[pre + "W3"])
    shared["kb1d"] = f32(inputs["kb1"]).reshape(L, H, 2, 128)
    shared["kb2d"] = f32(inputs["kb2"]).reshape(L, H, 2, 128)
    shared["kb3d"] = f32(inputs["kb3"]).reshape(L, H, AD, 1)
    shared["vb1d"] = f32(inputs["vb1"]).reshape(L, H, 2, 128)
    shared["vb2d"] = f32(inputs["vb2"]).reshape(L, H, 2, 128)
    shared["vb3r"] = np.ascontiguousarray(
        np.tile(f32(inputs["vb3"]), (1, 1, 16))).reshape(L, H, 1, P).astype(BF)

    for nm, key in (("dsw", "ds_W"), ("dew1", "de_W1"), ("dew2", "de_W2"),
                    ("dew3", "de_W3")):
        h, lo = _split(inputs[key])
        shared[nm + "h"] = h
        shared[nm + "l"] = lo
    h, lo = _split(f32(inputs["ds_b"]).reshape(1, D))
    shared["dsbh"], shared["dsbl"] = h, lo
    for nm, key, shp in (("ffw1", "ff_W1", None), ("ffw2", "ff_W2", None)):
        h, lo = _split(inputs[key])
        shared[nm + "h"], shared[nm + "l"] = h, lo
    for nm, key in (("ffb1", "ff_b1"), ("ffb2", "ff_b2")):
        h, lo = _split(f32(inputs[key]).reshape(L, 1, D))
        shared[nm + "h"], shared[nm + "l"] = h, lo
    for nm, key in (("deb1", "de_b1"), ("deb2", "de_b2"), ("deb3", "de_b3")):
        h, lo = _split(f32(inputs[key]).reshape(1, -1))
        shared[nm + "h"], shared[nm + "l"] = h, lo
    shared["ln1gd"] = f32(inputs["ln1_g"]).reshape(L, 2, 128, 1)
    shared["ln1bd"] = f32(inputs["ln1_b"]).reshape(L, 2, 128, 1)
    shared["ln2gd"] = f32(inputs["ln2_g"]).reshape(L, 2, 128, 1)
    shared["ln2bd"] = f32(inputs["ln2_b"]).reshape(L, 2, 128, 1)

    oh8f = np.zeros((8, D), np.float32)
    for hh in range(8):
        base = 128 * (hh // 4) + 32 * (hh % 4)
        oh8f[hh, base:base + 32] = 1.0
    shared["oh8d"] = oh8f
    shared["iotad"] = np.broadcast_to(
        np.arange(R, dtype=np.float32), (128, R)).copy()

    in_maps = []
    for b in range(B):
        merged = np.concatenate([enc[b], tu[b]], axis=1)  # [V, 257]
        mt = np.ascontiguousarray(merged.T)               # [257, V]
        cur = enc[b][pp_, :]                              # [P, I]
        curt = np.ascontiguousarray(cur.T)                # [I, P]
        ch, cl = _split(curt)
        m = dict(shared)
        m["xt0"] = mt[0:128].astype(BF)
        m["xt1"] = mt[128:256].astype(BF)
        m["xt2"] = mt[256:257].astype(BF)
        m["curh"], m["curl"] = ch, cl
        m["updc"] = tu[b][pp_, :]                          # [P,1] f32
        in_maps.append(m)

    trace = os.environ.get("KERNEL_TRACE") == "1" and _maybe_enable_trace()
    res = run_bass_kernel_spmd(
        nc, in_maps, core_ids=list(range(B)), trace=trace,
        trace_cores=list(range(B)) if trace else None)
    LAST_RESULT["res"] = res
    if trace and res.exec_time_ns is not None:
        print(f"HW exec time: {res.exec_time_ns} ns "
              f"(mean {res.mean_exec_time_ns} ns, "
              f"slowest core {res.max_exec_time_core_id})")
    out = np.array([res.results[b]["out"][0, 0] for b in range(B)], np.float32)
    return out
